# revision 5
# baseline (speedup 1.0000x reference)
"""Trainium2 Bass kernel for PoseSenceFlowModule (PointNet-style scene flow).

Sharding: program A runs the 4-level set-abstraction pyramid with one
(batch, cloud) unit per NeuronCore (4 batches x 2 clouds = 8 cores).
Program B runs cost-volume + upconv + heads with one (batch, query-half)
per core.  Host code only shards/reshapes inputs and gathers outputs.

Hardcoded problem shape: B=4, N=8192, f32.
"""

import numpy as np

import concourse.bass as bass
import concourse.bacc as bacc
import concourse.mybir as mybir
from concourse.tile import TileContext
from concourse.bass_utils import run_bass_kernel_spmd
from concourse.masks import make_identity

f32 = mybir.dt.float32
i16 = mybir.dt.int16
u16 = mybir.dt.uint16
AF = mybir.ActivationFunctionType
ALU = mybir.AluOpType
AX = mybir.AxisListType

B, N = 4, 8192

# level configs
LEVELS = [
    dict(nq=2048, nref=8192, k=32, qs=4, rs=1, tch=8, pg=16, l3=16,
         ncopies=8, chunk=256),
    dict(nq=1024, nref=2048, k=32, qs=8, rs=4, tch=16, pg=16, l3=32,
         ncopies=8, chunk=64),
    dict(nq=256, nref=1024, k=16, qs=32, rs=8, tch=32, pg=32, l3=64,
         ncopies=4, chunk=None),
    dict(nq=64, nref=256, k=16, qs=128, rs=32, tch=64, pg=64, l3=128,
         ncopies=2, chunk=None),
]


# ---------------------------------------------------------------- device utils

def topk_rows(nc, pool, D, rows, n, k, chunk, name, scr_tag=None):
    """Top-k (largest) per row of SBUF AP D [rows, n].
    Returns (vals [rows,k] f32 sorted desc, idx [rows,k] u16 positions)."""
    v = pool.tile([rows, k], f32, tag="tk_v")
    ix = pool.tile([rows, k], u16, tag="tk_ix")
    nrounds = k // 8
    if chunk is None:
        scr = pool.tile([rows, n], f32, tag=scr_tag or f"{name}_scr")
        nc.vector.tensor_copy(scr[:], D)
        src = scr[:]
    else:
        g = n // chunk
        cand = pool.tile([rows, g * 8], f32, tag="tk_cand")
        for c in range(g):
            nc.vector.max(out=cand[:, c * 8:c * 8 + 8],
                          in_=D.rearrange("p (g c) -> p g c", g=g)[:, c, :])
        scr = pool.tile([rows, g * 8], f32, tag="tk_cscr")
        nc.vector.tensor_copy(scr[:], cand[:])
        src = scr[:]
    for r in range(nrounds):
        nc.vector.max(out=v[:, r * 8:r * 8 + 8], in_=src)
        nc.vector.max_index(out=ix[:, r * 8:r * 8 + 8],
                            in_max=v[:, r * 8:r * 8 + 8], in_values=D)
        if r < nrounds - 1:
            nc.vector.match_replace(out=src, in_to_replace=v[:, r * 8:r * 8 + 8],
                                    in_values=src, imm_value=-1e30)
    return v, ix


def mm_evac(nc, psum_pool, sbuf_dst, lhsT, rhs, nfree, relu=False, bias=0.0,
            extra=None, exp=False):
    """lhsT.T @ rhs in N<=512 chunks (optional accumulating second pair in
    `extra`), ACT-evacuated into sbuf_dst with optional Relu/Exp + bias AP."""
    nchunks = (nfree + 511) // 512
    for c in range(nchunks):
        w = min(512, nfree - c * 512)
        ps = psum_pool.tile([sbuf_dst.shape[0], 512], f32, tag="mmps")
        nc.tensor.matmul(ps[:, :w], lhsT, rhs[:, c * 512:c * 512 + w],
                         start=True, stop=extra is None)
        if extra is not None:
            lhsT2, rhs2 = extra
            nc.tensor.matmul(ps[:, :w], lhsT2, rhs2[:, c * 512:c * 512 + w],
                             start=False, stop=True)
        func = AF.Relu if relu else (AF.Exp if exp else AF.Copy)
        nc.scalar.activation(sbuf_dst[:, c * 512:c * 512 + w], ps[:, :w],
                             func, bias=bias)


# ---------------------------------------------------------------- program A

def build_program_a(loop_n=1):
    nc = bacc.Bacc("TRN2", target_bir_lowering=False, debug=False, num_devices=8)

    def inp(name, shape, dt=f32):
        return nc.dram_tensor(name, shape, dt, kind="ExternalInput").ap()

    xyz_in = inp("xyzT", [3, N])
    col_in = inp("colT", [3, N])
    a4a_in = inp("a4a", [3, 4])
    a4b_in = inp("a4b", [3, 4])
    w1rep0b_in = inp("w1rep0b", [3, 128])
    w1rep = [inp(f"w1rep{l}", [3, 128]) for l in range(4)]
    wfrep = [inp(f"wfrep{l}", [LEVELS[l]["tch"], 128]) for l in range(1, 4)]
    cw = [inp(f"cw{l}", [3, LEVELS[l]["tch"]]) for l in range(4)]
    b1p = [inp(f"b1p{l}", [128, 1]) for l in range(4)]
    bd2 = [inp(f"bd2_{l}", [128, 128]) for l in range(4)]
    b2p = [inp(f"b2p{l}", [128, 1]) for l in range(4)]
    bd3 = [inp(f"bd3_{l}", [128, 128]) for l in range(4)]
    b3p = [inp(f"b3p{l}", [128, 1]) for l in range(4)]

    l2f_out = nc.dram_tensor("l2f", [64, 256], f32, kind="ExternalOutput").ap()
    l3f_out = nc.dram_tensor("l3f", [128, 64], f32, kind="ExternalOutput").ap()

    with TileContext(nc) as tc:
        with tc.tile_pool(name="const", bufs=1) as cpool, \
             tc.tile_pool(name="sb", bufs=1) as pool, \
             tc.tile_pool(name="ps", bufs=4, space="PSUM") as psp, \
             tc.tile_pool(name="ps2", bufs=2, space="PSUM") as psp2:

            ident = cpool.tile([128, 128], f32)
            make_identity(nc, ident[:])

            def body(_=None):
                xyzT = pool.tile([3, N], f32, tag="xyzT")
                colT = pool.tile([3, N], f32, tag="slotB")
                a4a = pool.tile([3, 4], f32, tag="a4a")
                a4b = pool.tile([3, 4], f32, tag="a4b")
                w1rep0b = pool.tile([3, 128], f32, tag="w1rep0b")
                nc.sync.dma_start(out=xyzT[:], in_=xyz_in[:, :])
                nc.sync.dma_start(out=colT[:], in_=col_in[:, :])
                nc.sync.dma_start(out=a4a[:], in_=a4a_in[:, :])
                nc.sync.dma_start(out=a4b[:], in_=a4b_in[:, :])
                nc.sync.dma_start(out=w1rep0b[:], in_=w1rep0b_in[:, :])
                consts = {}
                for l in range(4):
                    consts[l] = {}
                    for nm, src, shp in [
                        ("w1rep", w1rep[l], [3, 128]),
                        ("cw", cw[l], [3, LEVELS[l]["tch"]]),
                        ("b1p", b1p[l], [128, 1]), ("bd2", bd2[l], [128, 128]),
                        ("b2p", b2p[l], [128, 1]),
                        ("bd3", bd3[l], [128, 128]),
                        ("b3p", b3p[l], [128, 1]),
                    ]:
                        t = pool.tile(shp, f32, tag=f"c{nm}{l}")
                        nc.sync.dma_start(out=t[:], in_=src[:, :])
                        consts[l][nm] = t
                    if l > 0:
                        t = pool.tile([LEVELS[l]["tch"], 128], f32, tag=f"cwf{l}")
                        nc.sync.dma_start(out=t[:], in_=wfrep[l - 1][:, :])
                        consts[l]["wfrep"] = t

                # rhs4 = [2x;2y;2z;-r^2] for all 8192 ref points
                sq = pool.tile([3, N], f32, tag="slotA")
                nc.vector.tensor_tensor(sq[:], xyzT[:], xyzT[:], ALU.mult)
                rhs4 = pool.tile([4, N], f32, tag="rhs4")
                mm_evac(nc, psp, rhs4, a4a[:], xyzT[:], N, extra=(a4b[:], sq[:]))
                # lhsT4 = [x;y;z;1] for the 2048 level-0 query points
                lhsT4 = pool.tile([4, 2048], f32, tag="lhsT4")
                nc.vector.memset(lhsT4[:, :], 1.0)
                nc.vector.tensor_copy(lhsT4[0:3, :], xyzT[:, ::4])

                featT = None
                for l, cfg in enumerate(LEVELS):
                    nq, nref, k = cfg["nq"], cfg["nref"], cfg["k"]
                    qs, rs, tch, pg = cfg["qs"], cfg["rs"], cfg["tch"], cfg["pg"]
                    ncp, chunk = cfg["ncopies"], cfg["chunk"]
                    qpc = nq * pg // 128
                    cores_per_copy = pg // 16
                    nqt = (nq + 127) // 128
                    rows = min(nq, 128)

                    # ---- gather table [128, nref] (ncp replicated copies)
                    table = pool.tile([128, nref], f32, tag="table")
                    if l == 0:
                        mm_evac(nc, psp, table, consts[0]["w1rep"][:], xyzT[:],
                                nref, extra=(w1rep0b[:], colT[:]))
                    else:
                        mm_evac(nc, psp, table, consts[l]["w1rep"][:],
                                xyzT[:, ::rs], nref,
                                extra=(consts[l]["wfrep"][:], featT[:]))

                    # ---- beta (query offsets) packed [128, qpc]
                    cq = pool.tile([tch, nq], f32, tag="cq")
                    mm_evac(nc, psp, cq, consts[l]["cw"][:], xyzT[:, ::qs], nq)
                    beta = pool.tile([128, qpc], f32, tag="beta")
                    nc.vector.memset(beta[:], 0.0)
                    for g in range(ncp):
                        nc.sync.dma_start(out=beta[pg * g:pg * g + tch, :],
                                          in_=cq[:, qpc * g:qpc * (g + 1)])

                    # ---- KNN selection + repack into wrapped idx tile
                    wrap = pool.tile([128, qpc * k // 16], i16, tag="wrap")
                    D = pool.tile([rows, nref], f32, tag="slotA")
                    step = qs // 4
                    for t in range(nqt):
                        lq = lhsT4[:, t * 128 * step:(t * 128 + rows) * step:step]
                        for c in range((nref + 511) // 512):
                            w = min(512, nref - c * 512)
                            ps = psp.tile([rows, 512], f32, tag="mmps")
                            nc.tensor.matmul(
                                ps[:, :w], lq,
                                rhs4[:, c * 512 * rs:(c * 512 + w) * rs:rs],
                                start=True, stop=True)
                            nc.scalar.activation(D[:, c * 512:c * 512 + w],
                                                 ps[:, :w], AF.Copy, bias=0.0)
                        _, ix = topk_rows(nc, pool, D[:], rows, nref, k, chunk,
                                          f"tk{l}", scr_tag="cq")
                        ixf = pool.tile([rows, k], f32, tag="ixf")
                        nc.vector.tensor_copy(ixf[:], ix[:])
                        tp = psp2.tile([k, rows], f32, tag="tp")
                        nc.tensor.transpose(tp[:], ixf[:], ident[:rows, :rows])
                        ti = pool.tile([k, rows], i16, tag="ti")
                        nc.vector.tensor_copy(ti[:], tp[:])
                        # scatter transposed columns into per-core wrapped layout
                        ncpt = max(1, rows // qpc)      # copies per q-tile
                        colw = min(qpc, rows)
                        so = ((t * 128) % qpc) * k // 16
                        for g in range(ncpt):
                            cp = (t * 128) // qpc + g
                            cols = slice(g * colw, (g + 1) * colw)
                            for cc in range(cores_per_copy):
                                base = cp * pg + cc * 16
                                if k == 32:
                                    for h in range(2):
                                        nc.sync.dma_start(
                                            out=wrap[base:base + 16,
                                                     so + h:so + 2 * colw:2],
                                            in_=ti[16 * h:16 * h + 16, cols])
                                else:
                                    nc.sync.dma_start(
                                        out=wrap[base:base + 16, so:so + colw],
                                        in_=ti[0:16, cols])

                    # ---- gather + MLP + pool
                    G = pool.tile([128, qpc * k], f32, tag="slotB")
                    nc.gpsimd.ap_gather(out_ap=G[:], in_ap=table[:], idxs_ap=wrap[:],
                                        channels=128, num_elems=nref, d=1,
                                        num_idxs=qpc * k)
                    bb = beta[:].unsqueeze(2).to_broadcast([128, qpc, k])
                    g3 = G[:].rearrange("p (q k) -> p q k", k=k)
                    nc.vector.tensor_tensor(g3, g3, bb, ALU.add)
                    nc.scalar.activation(G[:], G[:], AF.Relu,
                                         bias=consts[l]["b1p"][:])
                    H2 = pool.tile([128, qpc * k], f32, tag="slotA")
                    mm_evac(nc, psp, H2, consts[l]["bd2"][:], G[:], qpc * k,
                            relu=True, bias=consts[l]["b2p"][:])
                    nparts = 1 if l == 0 else 2
                    pooled = []
                    for p_ in range(nparts):
                        H3 = pool.tile([128, qpc * k], f32, tag="slotB")
                        rhs_h = H2[:, :] if nparts == 1 else H2[64 * p_:64 * p_ + 64, :]
                        lhs3 = consts[l]["bd3"][:] if nparts == 1 else \
                            consts[l]["bd3"][64 * p_:64 * p_ + 64, :]
                        mm_evac(nc, psp, H3, lhs3, rhs_h, qpc * k,
                                relu=True, bias=consts[l]["b3p"][:])
                        pl = pool.tile([128, qpc], f32, tag=f"pool{p_}")
                        nc.vector.tensor_reduce(
                            out=pl[:], in_=H3[:].rearrange("p (q k) -> p q k", k=k),
                            op=ALU.max, axis=AX.X)
                        pooled.append(pl)

                    # ---- unpack pooled -> table-form featT [ch_out, nq]
                    ch_out = cfg["l3"]
                    nfT = pool.tile([ch_out, nq], f32, tag="feat")
                    if l == 0:
                        for g in range(8):
                            nc.sync.dma_start(out=nfT[0:16, 256 * g:256 * (g + 1)],
                                              in_=pooled[0][16 * g:16 * g + 16, :])
                    elif l == 1:
                        for p_ in range(2):
                            for g in range(4):
                                q0 = 128 * (4 * p_ + g)
                                nc.sync.dma_start(
                                    out=nfT[0:32, q0:q0 + 128],
                                    in_=pooled[p_][32 * g:32 * g + 32, :])
                    elif l == 2:
                        for p_ in range(2):
                            for g in range(2):
                                q0 = 64 * (2 * p_ + g)
                                nc.sync.dma_start(
                                    out=nfT[0:64, q0:q0 + 64],
                                    in_=pooled[p_][64 * g:64 * g + 64, :])
                    else:
                        for p_ in range(2):
                            nc.vector.tensor_copy(nfT[:, 32 * p_:32 * p_ + 32],
                                                  pooled[p_][:])
                    featT = nfT
                    if l == 2:
                        nc.sync.dma_start(out=l2f_out[:, :], in_=nfT[:])
                    if l == 3:
                        nc.sync.dma_start(out=l3f_out[:, :], in_=nfT[:])

            if loop_n > 1:
                with tc.For_i(0, loop_n, 1) as _i:
                    body()
            else:
                body()

    nc.compile()
    return nc


# ---------------------------------------------------------------- program B

def build_program_b(loop_n=1):
    nc = bacc.Bacc("TRN2", target_bir_lowering=False, debug=False, num_devices=8)

    def inp(name, shape, dt=f32):
        return nc.dram_tensor(name, shape, dt, kind="ExternalInput").ap()

    x1_in = inp("x1T", [3, 64]); x2_in = inp("x2T", [3, 64])
    a4a_in = inp("a4a", [3, 4]); a4b_in = inp("a4b", [3, 4])
    l2xh = inp("l2xh", [3, 128])
    f1 = inp("f1", [128, 64]); f2 = inp("f2", [128, 64])
    l2fh = inp("l2fh", [64, 128])
    tabx1_in = inp("tabx1", [128, 64]); tabx2_in = inp("tabx2", [128, 64])
    WN = {}
    for nm, shp in [
        ("cv1f2a", [128, 128]), ("cv1f2b", [128, 128]),
        ("cv1f1a", [128, 128]), ("cv1f1b", [128, 128]),
        ("cv1da", [3, 128]), ("cv1db", [3, 128]),
        ("ncv1da", [3, 128]), ("ncv1db", [3, 128]),
        ("cv1w1a", [128, 128]), ("cv1w1b", [128, 128]), ("cv1w2", [128, 128]),
        ("cvq1w", [3, 128]), ("cv2pca", [128, 128]), ("cv2pcb", [128, 128]),
        ("cv2da", [3, 128]), ("cv2db", [3, 128]),
        ("cv2w1a", [128, 128]), ("cv2w1b", [128, 128]), ("cvq2w", [3, 128]),
        ("upd", [3, 128]), ("upf", [128, 128]), ("upw1", [128, 128]),
        ("upw2a", [128, 128]), ("upw2b", [128, 128]),
        ("u2pa_a", [128, 128]), ("u2pb_a", [128, 128]), ("u2f_a", [64, 128]),
        ("u2pa_b", [128, 128]), ("u2pb_b", [128, 128]), ("u2f_b", [64, 128]),
        ("fpa", [128, 128]), ("fpb", [128, 128]), ("floww", [128, 3]),
        ("bcv1a", [128, 1]), ("bcv1b", [128, 1]), ("bcv11", [128, 1]),
        ("bcv12", [128, 1]), ("bq1", [128, 1]), ("bcv2a", [128, 1]),
        ("bcv2b", [128, 1]), ("bcv21", [128, 1]), ("bq2", [128, 1]),
        ("bup1", [128, 1]), ("bup2", [128, 1]), ("bup3a", [128, 1]),
        ("bup3b", [128, 1]), ("bu2a", [128, 1]), ("bu2b", [128, 1]),
        ("bfp", [128, 1]), ("bflow", [3, 1]),
    ]:
        WN[nm] = inp(nm, shp)

    out3 = nc.dram_tensor("out3", [3, 128], f32, kind="ExternalOutput").ap()

    with TileContext(nc) as tc:
        with tc.tile_pool(name="const", bufs=1) as cpool, \
             tc.tile_pool(name="sb", bufs=1) as pool, \
             tc.tile_pool(name="ps", bufs=4, space="PSUM") as psp, \
             tc.tile_pool(name="ps2", bufs=2, space="PSUM") as psp2:

            ident = cpool.tile([128, 128], f32)
            make_identity(nc, ident[:])

            def body(_=None):
                W = {}
                for nm, ap_in in WN.items():
                    t = pool.tile(list(ap_in.shape), f32, tag=f"W{nm}")
                    nc.sync.dma_start(out=t[:], in_=ap_in[:, :])
                    W[nm] = t
                xa = pool.tile([3, 64], f32, tag="xa")
                xb = pool.tile([3, 64], f32, tag="xb")
                sqa = pool.tile([3, 64], f32, tag="sqa")
                sqb = pool.tile([3, 64], f32, tag="sqb")
                a4a = pool.tile([3, 4], f32, tag="a4a")
                a4b = pool.tile([3, 4], f32, tag="a4b")
                nc.sync.dma_start(out=a4a[:], in_=a4a_in[:, :])
                nc.sync.dma_start(out=a4b[:], in_=a4b_in[:, :])
                l2x = pool.tile([3, 128], f32, tag="l2x")
                fa = pool.tile([128, 64], f32, tag="fa")
                fb = pool.tile([128, 64], f32, tag="fb")
                l2f = pool.tile([64, 128], f32, tag="l2f")
                tx1 = pool.tile([128, 64], f32, tag="tx1")
                tx2 = pool.tile([128, 64], f32, tag="tx2")
                for t_, s_ in [(xa, x1_in), (xb, x2_in), (l2x, l2xh), (fa, f1),
                               (fb, f2), (l2f, l2fh), (tx1, tabx1_in),
                               (tx2, tabx2_in)]:
                    nc.sync.dma_start(out=t_[:], in_=s_[:, :])

                nc.vector.tensor_tensor(sqa[:], xa[:], xa[:], ALU.mult)
                nc.vector.tensor_tensor(sqb[:], xb[:], xb[:], ALU.mult)
                rhs4_1 = pool.tile([4, 64], f32, tag="rhs4_1")
                rhs4_2 = pool.tile([4, 64], f32, tag="rhs4_2")
                mm_evac(nc, psp, rhs4_1, a4a[:], xa[:], 64, extra=(a4b[:], sqa[:]))
                mm_evac(nc, psp, rhs4_2, a4a[:], xb[:], 64, extra=(a4b[:], sqb[:]))
                lq3 = pool.tile([4, 64], f32, tag="lq3")
                nc.vector.memset(lq3[:, :], 1.0)
                nc.vector.tensor_copy(lq3[0:3, :], xa[:, :])
                lq2 = pool.tile([4, 128], f32, tag="lq2")
                nc.vector.memset(lq2[:, :], 1.0)
                nc.vector.tensor_copy(lq2[0:3, :], l2x[:])

                def knn(lq, rows, rhs4, nref, name):
                    Dt = pool.tile([rows, nref], f32, tag=f"D{name}")
                    mm_evac(nc, psp, Dt, lq, rhs4[:], nref)
                    v = pool.tile([rows, 8], f32, tag=f"v{name}")
                    ix = pool.tile([rows, 8], u16, tag=f"ix{name}")
                    nc.vector.max(out=v[:], in_=Dt[:])
                    nc.vector.max_index(out=ix[:], in_max=v[:], in_values=Dt[:])
                    ixf = pool.tile([rows, 8], f32, tag=f"ixf{name}")
                    nc.vector.tensor_copy(ixf[:], ix[:])
                    return ixf

                def repack(ixf, rows, kk, name):
                    tp = psp2.tile([8, rows], f32, tag="tpb")
                    nc.tensor.transpose(tp[:], ixf[:, :], ident[:rows, :rows])
                    ti = pool.tile([8, rows], i16, tag=f"tib{name}")
                    nc.vector.tensor_copy(ti[:], tp[:])
                    wr = pool.tile([128, rows * kk // 16], i16, tag=f"wr{name}")
                    for c in range(8):
                        if kk == 8:
                            for h in range(2):
                                nc.sync.dma_start(
                                    out=wr[16 * c + 8 * h:16 * c + 8 * h + 8, :],
                                    in_=ti[0:8, h::2])
                        elif kk == 4:
                            for u in range(4):
                                nc.sync.dma_start(
                                    out=wr[16 * c + 4 * u:16 * c + 4 * u + 4, :],
                                    in_=ti[0:4, u::4])
                    return wr

                def gather(tab, wr, nidx, name):
                    g = pool.tile([128, nidx], f32, tag=f"g{name}")
                    nc.gpsimd.ap_gather(out_ap=g[:], in_ap=tab[:], idxs_ap=wr[:],
                                        channels=128, num_elems=64, d=1,
                                        num_idxs=nidx)
                    return g

                # ---------------- cost volume stage 1 (k=6 padded to 8)
                ixq = knn(lq3[:], 64, rhs4_2, 64, "q")
                wq = repack(ixq, 64, 8, "q")
                Ta = pool.tile([128, 64], f32, tag="Ta")
                Tb = pool.tile([128, 64], f32, tag="Tb")
                mm_evac(nc, psp, Ta, W["cv1f2a"][:], fb[:], 64,
                        extra=(W["cv1da"][:], xb[:, :]))
                mm_evac(nc, psp, Tb, W["cv1f2b"][:], fb[:], 64,
                        extra=(W["cv1db"][:], xb[:, :]))
                Ba = pool.tile([128, 64], f32, tag="Ba")
                Bb = pool.tile([128, 64], f32, tag="Bb")
                mm_evac(nc, psp, Ba, W["cv1f1a"][:], fa[:], 64,
                        extra=(W["ncv1da"][:], xa[:, :]))
                mm_evac(nc, psp, Bb, W["cv1f1b"][:], fa[:], 64,
                        extra=(W["ncv1db"][:], xa[:, :]))
                Ga = gather(Ta, wq, 512, "a")
                Gb = gather(Tb, wq, 512, "b")
                Gx2 = gather(tx2, wq, 512, "x2")
                D3 = pool.tile([3, 512], f32, tag="D3")
                nc.vector.tensor_tensor(
                    D3[:].rearrange("p (q k) -> p q k", k=8),
                    Gx2[0:3, :].rearrange("p (q k) -> p q k", k=8),
                    xa[:, :].unsqueeze(2).to_broadcast([3, 64, 8]),
                    ALU.subtract)
                for (Gt, Bt, bt) in [(Ga, Ba, "bcv1a"), (Gb, Bb, "bcv1b")]:
                    nc.vector.tensor_tensor(
                        Gt[:].rearrange("p (q k) -> p q k", k=8),
                        Gt[:].rearrange("p (q k) -> p q k", k=8),
                        Bt[:].unsqueeze(2).to_broadcast([128, 64, 8]),
                        ALU.add)
                    nc.scalar.activation(Gt[:], Gt[:], AF.Relu, bias=W[bt][:])
                H2 = pool.tile([128, 512], f32, tag="H2b")
                mm_evac(nc, psp, H2, W["cv1w1a"][:], Ga[:], 512, relu=True,
                        bias=W["bcv11"][:], extra=(W["cv1w1b"][:], Gb[:]))
                H3 = pool.tile([128, 512], f32, tag="H3b")
                mm_evac(nc, psp, H3, W["cv1w2"][:], H2[:], 512, relu=True,
                        bias=W["bcv12"][:])
                E = pool.tile([128, 512], f32, tag="E")
                mm_evac(nc, psp, E, W["cvq1w"][:], D3[:], 512, exp=True,
                        bias=W["bq1"][:])
                nc.vector.memset(E[:].rearrange("p (q k) -> p q k", k=8)[:, :, 6:8], 0.0)
                S = pool.tile([128, 64], f32, tag="S")
                nc.vector.tensor_reduce(out=S[:], in_=E[:].rearrange("p (q k) -> p q k", k=8),
                                        op=ALU.add, axis=AX.X)
                nc.vector.reciprocal(out=S[:], in_=S[:])
                nc.vector.tensor_tensor(
                    E[:].rearrange("p (q k) -> p q k", k=8),
                    E[:].rearrange("p (q k) -> p q k", k=8),
                    S[:].unsqueeze(2).to_broadcast([128, 64, 8]), ALU.mult)
                nc.vector.tensor_tensor(E[:], E[:], H3[:], ALU.mult)
                pc = pool.tile([128, 64], f32, tag="pc")
                nc.vector.tensor_reduce(out=pc[:], in_=E[:].rearrange("p (q k) -> p q k", k=8),
                                        op=ALU.add, axis=AX.X)

                # ---------------- cost volume stage 2 (k=4)
                ixs = knn(lq3[:], 64, rhs4_1, 64, "s")
                ws = repack(ixs, 64, 4, "s")
                Tca = pool.tile([128, 64], f32, tag="Tca")
                Tcb = pool.tile([128, 64], f32, tag="Tcb")
                mm_evac(nc, psp, Tca, W["cv2pca"][:], pc[:], 64)
                mm_evac(nc, psp, Tcb, W["cv2pcb"][:], pc[:], 64)
                Gca = gather(Tca, ws, 256, "ca")
                Gcb = gather(Tcb, ws, 256, "cb")
                Gx1 = gather(tx1, ws, 256, "x1")
                D2 = pool.tile([3, 256], f32, tag="D2")
                nc.vector.tensor_tensor(
                    D2[:].rearrange("p (q k) -> p q k", k=4),
                    Gx1[0:3, :].rearrange("p (q k) -> p q k", k=4),
                    xa[:, :].unsqueeze(2).to_broadcast([3, 64, 4]),
                    ALU.subtract)
                for (Gt, wd, bt) in [(Gca, "cv2da", "bcv2a"), (Gcb, "cv2db", "bcv2b")]:
                    ps = psp.tile([128, 512], f32, tag="mmps")
                    nc.tensor.matmul(ps[:, :256], W[wd][:], D2[:], start=True, stop=True)
                    nc.vector.tensor_tensor(Gt[:], Gt[:], ps[:, :256], ALU.add)
                    nc.scalar.activation(Gt[:], Gt[:], AF.Relu, bias=W[bt][:])
                Hc2 = pool.tile([128, 256], f32, tag="Hc2")
                mm_evac(nc, psp, Hc2, W["cv2w1a"][:], Gca[:], 256, relu=True,
                        bias=W["bcv21"][:], extra=(W["cv2w1b"][:], Gcb[:]))
                E2 = pool.tile([128, 256], f32, tag="E2")
                mm_evac(nc, psp, E2, W["cvq2w"][:], D2[:], 256, exp=True,
                        bias=W["bq2"][:])
                S2 = pool.tile([128, 64], f32, tag="S2")
                nc.vector.tensor_reduce(out=S2[:], in_=E2[:].rearrange("p (q k) -> p q k", k=4),
                                        op=ALU.add, axis=AX.X)
                nc.vector.reciprocal(out=S2[:], in_=S2[:])
                nc.vector.tensor_tensor(
                    E2[:].rearrange("p (q k) -> p q k", k=4),
                    E2[:].rearrange("p (q k) -> p q k", k=4),
                    S2[:].unsqueeze(2).to_broadcast([128, 64, 4]), ALU.mult)
                nc.vector.tensor_tensor(E2[:], E2[:], Hc2[:], ALU.mult)
                cost3 = pool.tile([128, 64], f32, tag="cost3")
                nc.vector.tensor_reduce(out=cost3[:], in_=E2[:].rearrange("p (q k) -> p q k", k=4),
                                        op=ALU.add, axis=AX.X)

                # ---------------- upconv (128 queries, k=8 exact)
                ixu = knn(lq2[:], 128, rhs4_1, 64, "u")
                wu = repack(ixu, 128, 8, "u")
                Tu = pool.tile([128, 64], f32, tag="Tu")
                mm_evac(nc, psp, Tu, W["upf"][:], cost3[:], 64)
                Gu = gather(Tu, wu, 1024, "u")
                Gx1u = gather(tx1, wu, 1024, "x1u")
                D3u = pool.tile([3, 1024], f32, tag="D3u")
                nc.vector.tensor_tensor(
                    D3u[:].rearrange("p (q k) -> p q k", k=8),
                    Gx1u[0:3, :].rearrange("p (q k) -> p q k", k=8),
                    l2x[:].unsqueeze(2).to_broadcast([3, 128, 8]),
                    ALU.subtract)
                for c in range(2):
                    ps = psp.tile([128, 512], f32, tag="mmps")
                    nc.tensor.matmul(ps[:], W["upd"][:], D3u[:, c * 512:c * 512 + 512],
                                     start=True, stop=True)
                    nc.vector.tensor_tensor(Gu[:, c * 512:c * 512 + 512],
                                            Gu[:, c * 512:c * 512 + 512],
                                            ps[:], ALU.add)
                U1 = pool.tile([128, 1024], f32, tag="U1")
                nc.scalar.activation(U1[:], Gu[:], AF.Relu, bias=W["bup1"][:])
                U2 = pool.tile([128, 1024], f32, tag="U2")
                mm_evac(nc, psp, U2, W["upw1"][:], U1[:], 1024, relu=True,
                        bias=W["bup2"][:])
                P3 = []
                for p_, (wnm, bnm) in enumerate([("upw2a", "bup3a"), ("upw2b", "bup3b")]):
                    U3 = pool.tile([128, 1024], f32, tag="U3")
                    mm_evac(nc, psp, U3, W[wnm][:], U2[:], 1024, relu=True,
                            bias=W[bnm][:])
                    pl = pool.tile([128, 128], f32, tag=f"P3_{p_}")
                    nc.vector.tensor_reduce(out=pl[:],
                                            in_=U3[:].rearrange("p (q k) -> p q k", k=8),
                                            op=ALU.max, axis=AX.X)
                    P3.append(pl)

                # ---------------- up2 + fp + flow
                FF = []
                for o, (wpa, wpb, wf, bu) in enumerate([
                        ("u2pa_a", "u2pb_a", "u2f_a", "bu2a"),
                        ("u2pa_b", "u2pb_b", "u2f_b", "bu2b")]):
                    ps = psp.tile([128, 512], f32, tag="mmps")
                    nc.tensor.matmul(ps[:, :128], W[wpa][:], P3[0][:], start=True, stop=False)
                    nc.tensor.matmul(ps[:, :128], W[wpb][:], P3[1][:], start=False, stop=False)
                    nc.tensor.matmul(ps[:, :128], W[wf][:], l2f[:], start=False, stop=True)
                    Ft = pool.tile([128, 128], f32, tag=f"F{o}")
                    nc.scalar.activation(Ft[:], ps[:, :128], AF.Relu, bias=W[bu][:])
                    FF.append(Ft)
                ps = psp.tile([128, 512], f32, tag="mmps")
                nc.tensor.matmul(ps[:, :128], W["fpa"][:], FF[0][:], start=True, stop=False)
                nc.tensor.matmul(ps[:, :128], W["fpb"][:], FF[1][:], start=False, stop=True)
                FFt = pool.tile([128, 128], f32, tag="FFt")
                nc.scalar.activation(FFt[:], ps[:, :128], AF.Relu, bias=W["bfp"][:])
                ps2 = psp2.tile([3, 128], f32, tag="flowps")
                nc.tensor.matmul(ps2[:], W["floww"][:], FFt[:], start=True, stop=True)
                ot = pool.tile([3, 128], f32, tag="ot")
                nc.vector.tensor_scalar_add(ot[:], ps2[:], W["bflow"][:])
                nc.sync.dma_start(out=out3[:, :], in_=ot[:])

            if loop_n > 1:
                with tc.For_i(0, loop_n, 1) as _i:
                    body()
            else:
                body()

    nc.compile()
    return nc


# ---------------------------------------------------------------- host prep

def _np(x):
    return np.asarray(x, dtype=np.float32)


def blockdiag(w, nrep, pin, pout):
    cin, cout = w.shape
    out = np.zeros((nrep * pin, nrep * pout), np.float32)
    for g in range(nrep):
        out[g * pin:g * pin + cin, g * pout:g * pout + cout] = w
    return out


def replicate_cols(w, nrep, pout):
    k, cout = w.shape
    out = np.zeros((k, nrep * pout), np.float32)
    for g in range(nrep):
        out[:, g * pout:g * pout + cout] = w
    return out


def packed_bias(b, nrep, pout):
    out = np.zeros((nrep * pout, 1), np.float32)
    for g in range(nrep):
        out[g * pout:g * pout + len(b), 0] = b
    return out


A4A = np.array([[2, 0, 0, 0], [0, 2, 0, 0], [0, 0, 2, 0]], np.float32)
A4B = np.array([[0, 0, 0, -1], [0, 0, 0, -1], [0, 0, 0, -1]], np.float32)


def prep_a_const(params):
    p = {k: _np(v) for k, v in params.items()}
    const = {"a4a": A4A, "a4b": A4B}
    for l in range(4):
        w0 = p[f"sa{l}_w0"]
        cfg = LEVELS[l]
        ncp, pg = cfg["ncopies"], cfg["pg"]
        const[f"w1rep{l}"] = replicate_cols(w0[0:3], ncp, pg)
        if l == 0:
            const["w1rep0b"] = replicate_cols(w0[3:6], 8, 16)
        else:
            const[f"wfrep{l}"] = replicate_cols(w0[3:], ncp, pg)
        const[f"cw{l}"] = -w0[0:3, :].copy()
        const[f"b1p{l}"] = packed_bias(p[f"sa{l}_b0"], ncp, pg)
        const[f"bd2_{l}"] = blockdiag(p[f"sa{l}_w1"], 128 // pg, pg, pg)
        const[f"b2p{l}"] = packed_bias(p[f"sa{l}_b1"], ncp, pg)
        w2 = p[f"sa{l}_w2"]
        if l == 0:
            const["bd3_0"] = blockdiag(w2, 8, 16, 16)
            const["b3p0"] = packed_bias(p["sa0_b2"], 8, 16)
        else:
            half = blockdiag(w2, 64 // pg, pg, 2 * pg)
            const[f"bd3_{l}"] = np.vstack([half, half])
            const[f"b3p{l}"] = packed_bias(p[f"sa{l}_b2"], 128 // (2 * pg), 2 * pg)
    return const


def prep_b_const(params):
    p = {k: _np(v) for k, v in params.items()}
    cv1 = p["cv1_w0"]; cv2 = p["cv2_w0"]; up0 = p["up_w0"]; up2w = p["up2_w0"]
    return {
        "a4a": A4A, "a4b": A4B,
        "cv1f2a": cv1[0:128, 0:128].copy(), "cv1f2b": cv1[0:128, 128:256].copy(),
        "cv1f1a": cv1[128:256, 0:128].copy(), "cv1f1b": cv1[128:256, 128:256].copy(),
        "cv1da": cv1[256:259, 0:128].copy(), "cv1db": cv1[256:259, 128:256].copy(),
        "ncv1da": -cv1[256:259, 0:128], "ncv1db": -cv1[256:259, 128:256],
        "cv1w1a": p["cv1_w1"][0:128].copy(), "cv1w1b": p["cv1_w1"][128:256].copy(),
        "cv1w2": p["cv1_w2"], "cvq1w": p["cvq1_w0"],
        "cv2pca": cv2[0:128, 0:128].copy(), "cv2pcb": cv2[0:128, 128:256].copy(),
        "cv2da": cv2[128:131, 0:128].copy(), "cv2db": cv2[128:131, 128:256].copy(),
        "cv2w1a": p["cv2_w1"][0:128].copy(), "cv2w1b": p["cv2_w1"][128:256].copy(),
        "cvq2w": p["cvq2_w0"],
        "upd": up0[0:3].copy(), "upf": up0[3:131].copy(), "upw1": p["up_w1"],
        "upw2a": p["up_w2"][:, 0:128].copy(), "upw2b": p["up_w2"][:, 128:256].copy(),
        "u2pa_a": up2w[0:128, 0:128].copy(), "u2pb_a": up2w[128:256, 0:128].copy(),
        "u2f_a": up2w[256:320, 0:128].copy(),
        "u2pa_b": up2w[0:128, 128:256].copy(), "u2pb_b": up2w[128:256, 128:256].copy(),
        "u2f_b": up2w[256:320, 128:256].copy(),
        "fpa": p["fp_w0"][0:128].copy(), "fpb": p["fp_w0"][128:256].copy(),
        "floww": p["flow_w"],
        "bcv1a": p["cv1_b0"][0:128, None].copy(), "bcv1b": p["cv1_b0"][128:256, None].copy(),
        "bcv11": p["cv1_b1"][:, None], "bcv12": p["cv1_b2"][:, None],
        "bq1": p["cvq1_b0"][:, None],
        "bcv2a": p["cv2_b0"][0:128, None].copy(), "bcv2b": p["cv2_b0"][128:256, None].copy(),
        "bcv21": p["cv2_b1"][:, None], "bq2": p["cvq2_b0"][:, None],
        "bup1": p["up_b0"][:, None], "bup2": p["up_b1"][:, None],
        "bup3a": p["up_b2"][0:128, None].copy(), "bup3b": p["up_b2"][128:256, None].copy(),
        "bu2a": p["up2_b0"][0:128, None].copy(), "bu2b": p["up2_b0"][128:256, None].copy(),
        "bfp": p["fp_b0"][:, None], "bflow": p["flow_b"][:, None],
    }


def make_a_inputs(xyz1, xyz2, color1, color2, const):
    ins = []
    for b in range(B):
        for (xyz, col) in [(xyz1, color1), (xyz2, color2)]:
            xT = _np(xyz[b]).T.copy()
            cT = _np(col[b]).T.copy()
            d = dict(const)
            d["xyzT"] = xT
            d["colT"] = cT
            ins.append(d)
    return ins


def _xyztab(xT):
    t = np.zeros((128, xT.shape[1]), np.float32)
    for c in range(8):
        t[16 * c:16 * c + 3] = xT
    return t


def make_b_inputs(xyz1, xyz2, const_b, res_a):
    ins = []
    for b in range(B):
        x1T = _np(xyz1[b]).T
        x2T = _np(xyz2[b]).T
        l3x1T = np.ascontiguousarray(x1T[:, ::128][:, :64])
        l3x2T = np.ascontiguousarray(x2T[:, ::128][:, :64])
        l2x1T = x1T[:, ::32][:, :256]
        l2f1 = res_a[2 * b]["l2f"]
        l3f1 = res_a[2 * b]["l3f"]
        l3f2 = res_a[2 * b + 1]["l3f"]
        for h in range(2):
            d = dict(const_b)
            d["x1T"] = l3x1T
            d["x2T"] = l3x2T
            d["l2xh"] = np.ascontiguousarray(l2x1T[:, 128 * h:128 * h + 128])
            d["f1"] = l3f1; d["f2"] = l3f2
            d["l2fh"] = np.ascontiguousarray(l2f1[:, 128 * h:128 * h + 128])
            d["tabx1"] = _xyztab(l3x1T)
            d["tabx2"] = _xyztab(l3x2T)
            ins.append(d)
    return ins


_CACHE = {}


def kernel(xyz1, xyz2, color1, color2, params):
    xyz1 = _np(xyz1); xyz2 = _np(xyz2)
    color1 = _np(color1); color2 = _np(color2)
    if "a" not in _CACHE:
        _CACHE["a"] = build_program_a()
    if "b" not in _CACHE:
        _CACHE["b"] = build_program_b()
    nca, ncb = _CACHE["a"], _CACHE["b"]

    ins_a = make_a_inputs(xyz1, xyz2, color1, color2, prep_a_const(params))
    res_a = run_bass_kernel_spmd(nca, ins_a, list(range(8))).results
    ins_b = make_b_inputs(xyz1, xyz2, prep_b_const(params), res_a)
    res_b = run_bass_kernel_spmd(ncb, ins_b, list(range(8))).results

    out = np.zeros((B, 256, 3), np.float32)
    for b in range(B):
        for h in range(2):
            out[b, 128 * h:128 * h + 128, :] = res_b[2 * b + h]["out3"].T
    return out
